# revision 9
# baseline (speedup 1.0000x reference)
"""BiEncoder (bidirectional LSTM over video features) Trainium2 kernel — v4.

Sharding: 8 NeuronCores = 8 batch groups (B=32 each); EACH core runs BOTH
directions over its 32 rows. The embed (video @ W_e.T, the largest single
PE cost) is computed once per batch element instead of twice — no
cross-core communication needed. The two per-core scan chains (fwd/bwd)
are independent recurrences that interleave on every engine, hiding each
other's cross-engine latency, so the per-step chain no longer needs the
KH-half tail splitting of v3.

Backward-direction consumption runs over v in reverse time order, so the
full embedded v stays resident in SBUF (16KB/partition) and chunks are
embedded in the order {0,7},{1,6},{2,5},{3,4} — each pair just-in-time
for the window that consumes it from both ends.

Other structure carried over from v3:
  - No xg staging: each dir-step's input projection accumulates into its
    own one-bank gate PSUM, opened by a bias-broadcast identity matmul,
    emitted 2 steps ahead (6 pg banks + 2 embed banks = all of PSUM).
  - hh matmuls for i/f/o tiles in fp8e4m3 DoubleRow off an fp8 h copy;
    g tile fp16 (accuracy: fp8 g or any-phase-A-fp8 exceeds the 2e-2
    budget; ifo-hh-fp8 adds ~5e-3).
  - All gate tiles carry PSUM scale 512 (exact exponent shift in f16);
    one tanh(scale=1/512) covers i/g/f.
  - x2-state cell update (C=2c, H=2h; W_hh pre-halved, host halves the
    output) in fused scalar_tensor_tensor ops.
  - Gate tile order [i(0:4), g(4:8), f(8:12), o(12:16)].
"""

import sys
import time

for _p in ("/opt/trn_rl_repo", "/root/.axon_site/_ro/trn_rl_repo"):
    if _p not in sys.path:
        sys.path.insert(0, _p)

import numpy as np
import jax

try:
    jax.config.update("jax_compilation_cache_dir", "/tmp/jax_cc_cache")
    jax.config.update("jax_persistent_cache_min_entry_size_bytes", 0)
    jax.config.update("jax_persistent_cache_min_compile_time_secs", 0.0)
except Exception:
    pass

import concourse.tile as tile
from concourse import bacc, mybir
from concourse.bass import ts
from concourse.bass_utils import run_bass_kernel_spmd

F16 = mybir.dt.float16
F32 = mybir.dt.float32
F8 = mybir.dt.float8e4
F8_NP = mybir.dt.np(F8)
AF = mybir.ActivationFunctionType
OP = mybir.AluOpType
PM = mybir.MatmulPerfMode

B, T, F, P, H = 256, 64, 2048, 512, 512
NB = 8          # batch groups (one per core)
BC = B // NB    # 32 per-core batch
TC = 8
NCHUNK = T // TC
KF = F // 128
KP = P // 128
KH = H // 128
MG = 4 * H // 128  # 16 gate tiles: [i(0:4), g(4:8), f(8:12), o(12:16)]

IFO = list(range(16))  # ALL gate tiles fp8 for hh (incl. g: validated on
                       # the full batch against the true reference)
NI = len(IFO)

S_H = 16.0    # h8 = fp8(S_H * H), H = 2h
S_WH = 32.0   # whh8 = fp8(S_WH * Whh_eff)
S_G = S_H * S_WH  # 512; uniform gate PSUM scale

# embed order: window w consumes chunks w (fwd) and 7-w (bwd)
EMBED_ORDER = [0, 7, 1, 6, 2, 5, 3, 4]


def build_nc():
    nc = bacc.Bacc("TRN2", target_bir_lowering=False, debug=False, num_devices=8)

    vt_d = nc.dram_tensor("vt", [NCHUNK, 128, KF, TC, BC], F16, kind="ExternalInput")
    wet_d = nc.dram_tensor("w_et", [128, KF, P], F16, kind="ExternalInput")
    bet_d = nc.dram_tensor("b_e_t", [128, KP], F32, kind="ExternalInput")
    wihg_d = [nc.dram_tensor(f"w_ihg{d}", [128, KP, 4 * H], F16, kind="ExternalInput")
              for d in range(2)]
    whh8_d = [nc.dram_tensor(f"w_hh8{d}", [128, 2, NI, 2, 128], F8,
                             kind="ExternalInput") for d in range(2)]
    biasbc_d = [nc.dram_tensor(f"biasbc{d}", [128, MG, BC], F16,
                               kind="ExternalInput") for d in range(2)]
    ident_d = nc.dram_tensor("ident", [128, 128], F16, kind="ExternalInput")
    out_d = nc.dram_tensor("out_h", [2, NCHUNK, 128, TC, KH, BC], F16,
                           kind="ExternalOutput")

    with tile.TileContext(nc) as tc:
        with (
            tc.tile_pool(name="const", bufs=1) as const,
            tc.tile_pool(name="vload", bufs=2) as vload,
            tc.tile_pool(name="state", bufs=3) as state,
            tc.tile_pool(name="tmp", bufs=2) as tmp,
            tc.tile_pool(name="psv", bufs=2, space="PSUM") as psv,
            tc.tile_pool(name="psg", bufs=6, space="PSUM") as psg,
        ):
            # embed-critical consts first; the big scan weights are DMA'd
            # after the first two video chunks (below) so the prologue
            # embeds aren't stuck behind ~10MB of weight traffic.
            wet = const.tile([128, KF, P], F16)
            nc.sync.dma_start(wet[:], wet_d.ap())
            bet = const.tile([128, KP], F32)
            nc.sync.dma_start(bet[:], bet_d.ap())
            ident = const.tile([128, 128], F16)
            nc.sync.dma_start(ident[:], ident_d.ap())
            wihg, whh8, biasbc = [], [], []

            def load_scan_weights():
                # prefill consumers (wihg, biasbc) first, recurrence weights
                # (whhg, whh8) after — they are needed ~2us later
                for d in range(2):
                    w1 = const.tile([128, KP, 4 * H], F16, name=f"wihg{d}")
                    nc.sync.dma_start(w1[:], wihg_d[d].ap())
                    wihg.append(w1)
                    w4 = const.tile([128, MG, BC], F16, name=f"biasbc{d}")
                    nc.sync.dma_start(w4[:], biasbc_d[d].ap())
                    biasbc.append(w4)
                for d in range(2):
                    w3 = const.tile([128, 2, NI, 2, 128], F8, name=f"whh8{d}")
                    nc.sync.dma_start(w3[:], whh8_d[d].ap())
                    whh8.append(w3)

            # whole embedded sequence stays resident (bwd reads it reversed)
            vsb = const.tile([128, KP, T, BC], F16)

            h_prev, h8_prev, c_prev = [], [], []
            for d in range(2):
                hp = state.tile([128, KH, BC], F16, tag=f"h{d}")
                nc.gpsimd.memset(hp[:], 0.0)
                h8p = state.tile([128, KH, BC], F8, tag=f"h8{d}")
                nc.gpsimd.memset(h8p[:], 0.0)
                cp = state.tile([128, KH, BC], F16, tag=f"c{d}")
                nc.gpsimd.memset(cp[:], 0.0)
                h_prev.append(hp)
                h8_prev.append(h8p)
                c_prev.append(cp)

            def phase_a_items(c):
                """Embed chunk c into the resident vsb[:, :, c*TC:(c+1)*TC, :]."""
                vch = vload.tile([128, KF, TC * BC], F16, tag="vch")

                def dma_item():
                    nc.sync.dma_start(
                        vch[:], vt_d.ap()[c].rearrange("p ko t b -> p ko (t b)")
                    )

                def embed_item(mp):
                    pv = psv.tile([128, TC * BC], F32, tag="pv")
                    for ko in range(KF):
                        nc.tensor.matmul(
                            pv[:],
                            wet[:, ko, ts(mp, 128)],
                            vch[:, ko, :],
                            start=(ko == 0),
                            stop=(ko == KF - 1),
                        )
                    nc.scalar.activation(
                        vsb[:, mp, c * TC : (c + 1) * TC, :]
                        .rearrange("p t b -> p (t b)"),
                        pv[:],
                        AF.Identity,
                        bias=bet[:, mp : mp + 1],
                    )

                return [dma_item] + [lambda mp=mp: embed_item(mp) for mp in range(KP)]

            def emit_gate_prefill(d, pg, t):
                # v-time: fwd consumes t, bwd consumes T-1-t
                vt_time = t if d == 0 else T - 1 - t
                nc.tensor.matmul(
                    pg[:, :, :],
                    ident[:, :],
                    biasbc[d][:, :, :],
                    start=True,
                    stop=False,
                    skip_group_check=True,
                )
                for m in range(MG):
                    for kp in range(KP):
                        nc.tensor.matmul(
                            pg[:, m, :],
                            wihg[d][:, kp, ts(m, 128)],
                            vsb[:, kp, vt_time, :],
                            start=False,
                            stop=False,
                            skip_group_check=True,
                        )

            def scan_step(d, pg, tl, hstage):
                """Generator: yields at engine-stage boundaries so the two
                directions' same-stage ops can be emitted adjacently."""
                th = tmp.tile([128, MG, BC], F16, tag=f"th{d}")

                def mm_g():
                    for kh in range(KH):
                        for gi, m in enumerate(GT):
                            nc.tensor.matmul(
                                pg[:, m, :],
                                whhg[d][:, kh, ts(gi, 128)],
                                h_prev[d][:, kh, :],
                                start=False,
                                stop=False,
                                skip_group_check=True,
                            )

                def mm_8(tiles):
                    for pr in range(2):
                        for m in tiles:
                            mi = IFO.index(m)
                            nc.tensor.matmul(
                                pg[:, m, :],
                                whh8[d][:, pr, mi, :, :],
                                h8_prev[d][:, 2 * pr : 2 * pr + 2, :],
                                start=False,
                                stop=False,
                                perf_mode=PM.DoubleRow,
                                skip_group_check=True,
                            )

                mm_8(range(MG))  # all gates fp8 DoubleRow
                yield
                nc.scalar.activation(
                    th[:, 0:12, :], pg[:, 0:12, :], AF.Tanh, scale=1.0 / S_G
                )
                yield
                m2 = tmp.tile([128, KH, BC], F16, tag=f"m2{d}")
                nc.vector.scalar_tensor_tensor(
                    m2[:], th[:, 0:4, :], 1.0, th[:, 4:8, :], OP.add, OP.mult
                )
                m1 = tmp.tile([128, KH, BC], F16, tag=f"m1{d}")
                nc.vector.scalar_tensor_tensor(
                    m1[:], th[:, 8:12, :], 1.0, c_prev[d][:], OP.add, OP.mult
                )
                yield
                nc.scalar.activation(
                    th[:, 12:16, :], pg[:, 12:16, :], AF.Tanh, scale=1.0 / S_G
                )
                yield
                c_new = state.tile([128, KH, BC], F16, tag=f"c{d}")
                nc.vector.scalar_tensor_tensor(
                    c_new[:], m1[:], 0.5, m2[:], OP.mult, OP.add
                )
                yield
                tc_t = tmp.tile([128, KH, BC], F16, tag=f"tct{d}")
                nc.scalar.activation(tc_t[:], c_new[:], AF.Tanh, scale=0.5)
                yield
                h8_new = state.tile([128, KH, BC], F8, tag=f"h8{d}")
                nc.vector.scalar_tensor_tensor(
                    h8_new[:], th[:, 12:16, :], 1.0, tc_t[:], OP.add, OP.mult
                )
                h_new = hstage[:, tl, :, :]
                nc.vector.scalar_tensor_tensor(
                    h_new, th[:, 12:16, :], 1.0, tc_t[:], OP.add, OP.mult
                )
                h_prev[d], h8_prev[d], c_prev[d] = h_new, h8_new, c_new

            # ---- emission ----
            LOOKAHEAD = 2
            from collections import deque

            # prologue: embed chunks 0 and 7, then prefills for steps 0,1.
            # Video DMAs go first, then the scan weights, then the embeds.
            pro0 = phase_a_items(EMBED_ORDER[0])
            pro1 = phase_a_items(EMBED_ORDER[1])
            pro0[0]()
            pro1[0]()
            load_scan_weights()
            for it in pro0[1:] + pro1[1:]:
                it()
            pg_q = [deque(), deque()]
            for glob in range(LOOKAHEAD):
                for d in range(2):
                    pg = psg.tile([128, MG, BC], F32, tag="pg")
                    emit_gate_prefill(d, pg, glob)
                    pg_q[d].append(pg)

            items, n_items, emitted = [], 0, 0
            hstages = [None, None]
            for glob in range(T):
                c, tl = divmod(glob, TC)
                if tl == 0:
                    for d in range(2):
                        hst = state.tile(
                            [128, TC, KH, BC], F16, tag=f"hs{d}", name=f"hs{d}"
                        )
                        hstages[d] = hst
                    # embed the pair for window c+1 during window c
                    pos = 2 * (c + 1)
                    items = []
                    if pos < NCHUNK:
                        items = phase_a_items(EMBED_ORDER[pos])
                        items += phase_a_items(EMBED_ORDER[pos + 1])
                    n_items, emitted = len(items), 0
                gens = [
                    scan_step(d, pg_q[d].popleft(), tl, hstages[d])
                    for d in range(2)
                ]
                live = list(gens)
                while live:
                    live = [g for g in live if next(g, StopIteration) is None]
                nxt = glob + LOOKAHEAD
                if nxt < T:
                    for d in range(2):
                        pg = psg.tile([128, MG, BC], F32, tag="pg")
                        emit_gate_prefill(d, pg, nxt)
                        pg_q[d].append(pg)
                if n_items:
                    want = min(n_items, (n_items * (tl + 2)) // (TC - 1))
                    while emitted < want:
                        items[emitted]()
                        emitted += 1
                if tl == TC - 1:
                    for d in range(2):
                        nc.sync.dma_start(out_d.ap()[d, c], hstages[d][:])

    nc.compile()
    return nc


_CACHED_NC = None


def _get_nc():
    global _CACHED_NC
    if _CACHED_NC is None:
        _CACHED_NC = build_nc()
    return _CACHED_NC


def _prep_inputs(video_feats, W_e, b_e, W_ih1, W_hh1, b_ih1, b_hh1,
                 W_ih2, W_hh2, b_ih2, b_hh2):
    # gate row scaling (sigmoid-via-tanh): i,f,o rows 0.5; g rows 1.0
    s = np.ones((4 * H,), np.float32)
    s[0 * H : 2 * H] = 0.5
    s[3 * H : 4 * H] = 0.5
    perm = np.concatenate(
        [
            np.arange(0 * H, 1 * H),  # i
            np.arange(2 * H, 3 * H),  # g
            np.arange(1 * H, 2 * H),  # f
            np.arange(3 * H, 4 * H),  # o
        ]
    )

    wet = np.ascontiguousarray(
        W_e.T.astype(np.float16).reshape(KF, 128, P).transpose(1, 0, 2)
    )
    bet = np.ascontiguousarray(b_e.reshape(KP, 128).T).astype(np.float32)

    def pack8(Wt, scale):
        Wq = (Wt * scale).astype(F8_NP)
        out = np.zeros((128, 2, NI, 2, 128), F8_NP)
        for mi, m in enumerate(IFO):
            blk = Wq[:, m * 128 : (m + 1) * 128]
            for pr in range(Wt.shape[0] // 256):
                for j in range(2):
                    kt = 2 * pr + j
                    out[:, pr, mi, j, :] = blk[kt * 128 : (kt + 1) * 128, :]
        return np.ascontiguousarray(out)

    per_dir = []
    for (W_ih, W_hh, b_ih, b_hh) in (
        (W_ih1, W_hh1, b_ih1, b_hh1),
        (W_ih2, W_hh2, b_ih2, b_hh2),
    ):
        wih_eff = ((W_ih * s[:, None])[perm]).T.astype(np.float32)
        whh_eff = ((W_hh * s[:, None] * 0.5)[perm]).T.astype(np.float32)
        bb = (((b_ih + b_hh) * s)[perm]).astype(np.float32)
        biasbc = np.broadcast_to(
            (np.ascontiguousarray(bb.reshape(MG, 128).T) * S_G)[:, :, None],
            (128, MG, BC),
        ).astype(np.float16)
        wihg = np.ascontiguousarray(
            (wih_eff * S_G).astype(np.float16)
            .reshape(KP, 128, 4 * H)
            .transpose(1, 0, 2)
        )
        whh8 = pack8(whh_eff, S_G)
        per_dir.append((wihg, whh8, np.ascontiguousarray(biasbc)))

    vt_full = np.ascontiguousarray(video_feats.transpose(2, 1, 0)).astype(np.float16)

    in_maps = []
    for core in range(8):
        vt = np.ascontiguousarray(
            vt_full[:, :, core * BC : (core + 1) * BC]
            .reshape(KF, 128, NCHUNK, TC, BC)
            .transpose(2, 1, 0, 3, 4)
        )
        im = {
            "vt": vt,
            "w_et": wet,
            "b_e_t": bet,
            "ident": np.eye(128, dtype=np.float16),
        }
        for d in range(2):
            wihg, whh8, biasbc = per_dir[d]
            im[f"w_ihg{d}"] = wihg
            im[f"w_hh8{d}"] = whh8
            im[f"biasbc{d}"] = biasbc
        in_maps.append(im)
    return in_maps


last_exec_ns = None
last_wall_s = None


def kernel(**inputs):
    global last_exec_ns, last_wall_s
    nc = _get_nc()
    inputs = {k: np.asarray(v, dtype=np.float32) for k, v in inputs.items()}
    in_maps = _prep_inputs(**inputs)
    t0 = time.perf_counter()
    res = run_bass_kernel_spmd(nc, in_maps, core_ids=list(range(8)))
    last_wall_s = time.perf_counter() - t0
    last_exec_ns = res.exec_time_ns

    lstm1 = np.empty((B, T, H), np.float32)
    lstm2 = np.empty((B, T, H), np.float32)
    for core in range(8):
        oh = res.results[core]["out_h"]  # [2, NCHUNK, 128, TC, KH, BC], holds 2h
        for d, dst in ((0, lstm1), (1, lstm2)):
            h = np.transpose(
                oh[d].astype(np.float32), (4, 0, 2, 3, 1)
            ).reshape(BC, T, H)
            h *= 0.5
            if d == 1:
                h = h[:, ::-1, :]
            dst[core * BC : (core + 1) * BC] = h
    return (lstm1, lstm2)


# revision 10
# speedup vs baseline: 1.0098x; 1.0098x over previous
"""BiEncoder (bidirectional LSTM over video features) Trainium2 kernel — v4.

Sharding: 8 NeuronCores = 8 batch groups (B=32 each); EACH core runs BOTH
directions over its 32 rows. The embed (video @ W_e.T, the largest single
PE cost) is computed once per batch element instead of twice — no
cross-core communication needed. The two per-core scan chains (fwd/bwd)
are independent recurrences that interleave on every engine, hiding each
other's cross-engine latency, so the per-step chain no longer needs the
KH-half tail splitting of v3.

Backward-direction consumption runs over v in reverse time order, so the
full embedded v stays resident in SBUF (16KB/partition) and chunks are
embedded in the order {0,7},{1,6},{2,5},{3,4} — each pair just-in-time
for the window that consumes it from both ends.

Other structure carried over from v3:
  - No xg staging: each dir-step's input projection accumulates into its
    own one-bank gate PSUM, opened by a bias-broadcast identity matmul,
    emitted 2 steps ahead (6 pg banks + 2 embed banks = all of PSUM).
  - hh matmuls for i/f/o tiles in fp8e4m3 DoubleRow off an fp8 h copy;
    g tile fp16 (accuracy: fp8 g or any-phase-A-fp8 exceeds the 2e-2
    budget; ifo-hh-fp8 adds ~5e-3).
  - All gate tiles carry PSUM scale 512 (exact exponent shift in f16);
    one tanh(scale=1/512) covers i/g/f.
  - x2-state cell update (C=2c, H=2h; W_hh pre-halved, host halves the
    output) in fused scalar_tensor_tensor ops.
  - Gate tile order [i(0:4), g(4:8), f(8:12), o(12:16)].
"""

import sys
import time

for _p in ("/opt/trn_rl_repo", "/root/.axon_site/_ro/trn_rl_repo"):
    if _p not in sys.path:
        sys.path.insert(0, _p)

import numpy as np
import jax

try:
    jax.config.update("jax_compilation_cache_dir", "/tmp/jax_cc_cache")
    jax.config.update("jax_persistent_cache_min_entry_size_bytes", 0)
    jax.config.update("jax_persistent_cache_min_compile_time_secs", 0.0)
except Exception:
    pass

import concourse.tile as tile
from concourse import bacc, mybir
from concourse.bass import ts
from concourse.bass_utils import run_bass_kernel_spmd

F16 = mybir.dt.float16
F32 = mybir.dt.float32
F8 = mybir.dt.float8e4
F8_NP = mybir.dt.np(F8)
AF = mybir.ActivationFunctionType
OP = mybir.AluOpType
PM = mybir.MatmulPerfMode

B, T, F, P, H = 256, 64, 2048, 512, 512
NB = 8          # batch groups (one per core)
BC = B // NB    # 32 per-core batch
TC = 8
NCHUNK = T // TC
KF = F // 128
KP = P // 128
KH = H // 128
MG = 4 * H // 128  # 16 gate tiles: [i(0:4), g(4:8), f(8:12), o(12:16)]

IFO = list(range(16))  # ALL gate tiles fp8 for hh (incl. g: validated on
                       # the full batch against the true reference)
NI = len(IFO)

S_H = 16.0    # h8 = fp8(S_H * H), H = 2h
S_WH = 32.0   # whh8 = fp8(S_WH * Whh_eff)
S_G = S_H * S_WH  # 512; uniform gate PSUM scale

# embed order: window w consumes chunks w (fwd) and 7-w (bwd)
EMBED_ORDER = [0, 7, 1, 6, 2, 5, 3, 4]


def build_nc():
    nc = bacc.Bacc("TRN2", target_bir_lowering=False, debug=False, num_devices=8)

    vt_d = nc.dram_tensor("vt", [NCHUNK, 128, KF, TC, BC], F16, kind="ExternalInput")
    wet_d = nc.dram_tensor("w_et", [128, KF, P], F16, kind="ExternalInput")
    bet_d = nc.dram_tensor("b_e_t", [128, KP], F32, kind="ExternalInput")
    wihg_d = [nc.dram_tensor(f"w_ihg{d}", [128, KP, 4 * H], F16, kind="ExternalInput")
              for d in range(2)]
    whh8_d = [nc.dram_tensor(f"w_hh8{d}", [128, 2, NI, 2, 128], F8,
                             kind="ExternalInput") for d in range(2)]
    biasbc_d = [nc.dram_tensor(f"biasbc{d}", [128, MG, BC], F16,
                               kind="ExternalInput") for d in range(2)]
    ident_d = nc.dram_tensor("ident", [128, 128], F16, kind="ExternalInput")
    out_d = nc.dram_tensor("out_h", [2, NCHUNK, 128, TC, KH, BC], F16,
                           kind="ExternalOutput")

    with tile.TileContext(nc) as tc:
        with (
            tc.tile_pool(name="const", bufs=1) as const,
            tc.tile_pool(name="vload", bufs=2) as vload,
            tc.tile_pool(name="state", bufs=3) as state,
            tc.tile_pool(name="tmp", bufs=2) as tmp,
            tc.tile_pool(name="psv", bufs=2, space="PSUM") as psv,
            tc.tile_pool(name="psg", bufs=6, space="PSUM") as psg,
        ):
            # embed-critical consts first; the big scan weights are DMA'd
            # after the first two video chunks (below) so the prologue
            # embeds aren't stuck behind ~10MB of weight traffic.
            wet = const.tile([128, KF, P], F16)
            nc.sync.dma_start(wet[:], wet_d.ap())
            bet = const.tile([128, KP], F32)
            nc.sync.dma_start(bet[:], bet_d.ap())
            ident = const.tile([128, 128], F16)
            nc.sync.dma_start(ident[:], ident_d.ap())
            wihg, whh8, biasbc = [], [], []

            def load_scan_weights():
                # prefill consumers (wihg, biasbc) first, recurrence weights
                # (whhg, whh8) after — they are needed ~2us later
                for d in range(2):
                    w1 = const.tile([128, KP, 4 * H], F16, name=f"wihg{d}")
                    nc.sync.dma_start(w1[:], wihg_d[d].ap())
                    wihg.append(w1)
                    w4 = const.tile([128, MG, BC], F16, name=f"biasbc{d}")
                    nc.sync.dma_start(w4[:], biasbc_d[d].ap())
                    biasbc.append(w4)
                for d in range(2):
                    w3 = const.tile([128, 2, NI, 2, 128], F8, name=f"whh8{d}")
                    nc.sync.dma_start(w3[:], whh8_d[d].ap())
                    whh8.append(w3)

            # whole embedded sequence stays resident (bwd reads it reversed)
            vsb = const.tile([128, KP, T, BC], F16)

            h_prev, h8_prev, c_prev = [], [], []
            for d in range(2):
                hp = state.tile([128, KH, BC], F16, tag=f"h{d}")
                nc.gpsimd.memset(hp[:], 0.0)
                h8p = state.tile([128, KH, BC], F8, tag=f"h8{d}")
                nc.gpsimd.memset(h8p[:], 0.0)
                cp = state.tile([128, KH, BC], F16, tag=f"c{d}")
                nc.gpsimd.memset(cp[:], 0.0)
                h_prev.append(hp)
                h8_prev.append(h8p)
                c_prev.append(cp)

            def phase_a_items(c):
                """Embed chunk c into the resident vsb[:, :, c*TC:(c+1)*TC, :]."""
                vch = vload.tile([128, KF, TC * BC], F16, tag="vch")

                def dma_item():
                    nc.sync.dma_start(
                        vch[:], vt_d.ap()[c].rearrange("p ko t b -> p ko (t b)")
                    )

                def embed_item(mp):
                    pv = psv.tile([128, TC * BC], F32, tag="pv")
                    for ko in range(KF):
                        nc.tensor.matmul(
                            pv[:],
                            wet[:, ko, ts(mp, 128)],
                            vch[:, ko, :],
                            start=(ko == 0),
                            stop=(ko == KF - 1),
                        )
                    nc.scalar.activation(
                        vsb[:, mp, c * TC : (c + 1) * TC, :]
                        .rearrange("p t b -> p (t b)"),
                        pv[:],
                        AF.Identity,
                        bias=bet[:, mp : mp + 1],
                    )

                return [dma_item] + [lambda mp=mp: embed_item(mp) for mp in range(KP)]

            def emit_gate_prefill(d, pg, t):
                # v-time: fwd consumes t, bwd consumes T-1-t
                vt_time = t if d == 0 else T - 1 - t
                nc.tensor.matmul(
                    pg[:, :, :],
                    ident[:, :],
                    biasbc[d][:, :, :],
                    start=True,
                    stop=False,
                    skip_group_check=True,
                )
                for m in range(MG):
                    for kp in range(KP):
                        nc.tensor.matmul(
                            pg[:, m, :],
                            wihg[d][:, kp, ts(m, 128)],
                            vsb[:, kp, vt_time, :],
                            start=False,
                            stop=False,
                            skip_group_check=True,
                        )

            def scan_step(d, pg, tl, hstage):
                """Generator: yields at engine-stage boundaries so the two
                directions' same-stage ops can be emitted adjacently."""
                th = tmp.tile([128, MG, BC], F16, tag=f"th{d}")

                def mm_g():
                    for kh in range(KH):
                        for gi, m in enumerate(GT):
                            nc.tensor.matmul(
                                pg[:, m, :],
                                whhg[d][:, kh, ts(gi, 128)],
                                h_prev[d][:, kh, :],
                                start=False,
                                stop=False,
                                skip_group_check=True,
                            )

                def mm_8(tiles):
                    for pr in range(2):
                        for m in tiles:
                            mi = IFO.index(m)
                            nc.tensor.matmul(
                                pg[:, m, :],
                                whh8[d][:, pr, mi, :, :],
                                h8_prev[d][:, 2 * pr : 2 * pr + 2, :],
                                start=False,
                                stop=False,
                                perf_mode=PM.DoubleRow,
                                skip_group_check=True,
                            )

                mm_8(range(MG))  # all gates fp8 DoubleRow
                yield
                nc.scalar.activation(
                    th[:, 0:12, :], pg[:, 0:12, :], AF.Tanh, scale=1.0 / S_G
                )
                yield
                m2 = tmp.tile([128, KH, BC], F16, tag=f"m2{d}")
                nc.vector.scalar_tensor_tensor(
                    m2[:], th[:, 0:4, :], 1.0, th[:, 4:8, :], OP.add, OP.mult
                )
                m1 = tmp.tile([128, KH, BC], F16, tag=f"m1{d}")
                nc.vector.scalar_tensor_tensor(
                    m1[:], th[:, 8:12, :], 1.0, c_prev[d][:], OP.add, OP.mult
                )
                yield
                nc.scalar.activation(
                    th[:, 12:16, :], pg[:, 12:16, :], AF.Tanh, scale=1.0 / S_G
                )
                yield
                c_new = state.tile([128, KH, BC], F16, tag=f"c{d}")
                nc.vector.scalar_tensor_tensor(
                    c_new[:], m1[:], 0.5, m2[:], OP.mult, OP.add
                )
                yield
                tc_t = tmp.tile([128, KH, BC], F16, tag=f"tct{d}")
                nc.scalar.activation(tc_t[:], c_new[:], AF.Tanh, scale=0.5)
                yield
                h8_new = state.tile([128, KH, BC], F8, tag=f"h8{d}")
                for hv in range(2):
                    sl = slice(2 * hv, 2 * hv + 2)
                    nc.vector.scalar_tensor_tensor(
                        h8_new[:, sl, :], th[:, 12 + 2 * hv : 14 + 2 * hv, :],
                        1.0, tc_t[:, sl, :], OP.add, OP.mult,
                    )
                h_new = hstage[:, tl, :, :]
                nc.vector.scalar_tensor_tensor(
                    h_new, th[:, 12:16, :], 1.0, tc_t[:], OP.add, OP.mult
                )
                h_prev[d], h8_prev[d], c_prev[d] = h_new, h8_new, c_new

            # ---- emission ----
            LOOKAHEAD = 2
            from collections import deque

            # prologue: embed chunks 0 and 7, then prefills for steps 0,1.
            # Video DMAs go first, then the scan weights, then the embeds.
            pro0 = phase_a_items(EMBED_ORDER[0])
            pro1 = phase_a_items(EMBED_ORDER[1])
            pro0[0]()
            pro1[0]()
            load_scan_weights()
            for it in pro0[1:] + pro1[1:]:
                it()
            pg_q = [deque(), deque()]
            for glob in range(LOOKAHEAD):
                for d in range(2):
                    pg = psg.tile([128, MG, BC], F32, tag="pg")
                    emit_gate_prefill(d, pg, glob)
                    pg_q[d].append(pg)

            items, n_items, emitted = [], 0, 0
            hstages = [None, None]
            for glob in range(T):
                c, tl = divmod(glob, TC)
                if tl == 0:
                    for d in range(2):
                        hst = state.tile(
                            [128, TC, KH, BC], F16, tag=f"hs{d}", name=f"hs{d}"
                        )
                        hstages[d] = hst
                    # embed the pair for window c+1 during window c
                    pos = 2 * (c + 1)
                    items = []
                    if pos < NCHUNK:
                        items = phase_a_items(EMBED_ORDER[pos])
                        items += phase_a_items(EMBED_ORDER[pos + 1])
                    n_items, emitted = len(items), 0
                gens = [
                    scan_step(d, pg_q[d].popleft(), tl, hstages[d])
                    for d in range(2)
                ]
                live = list(gens)
                while live:
                    live = [g for g in live if next(g, StopIteration) is None]
                nxt = glob + LOOKAHEAD
                if nxt < T:
                    for d in range(2):
                        pg = psg.tile([128, MG, BC], F32, tag="pg")
                        emit_gate_prefill(d, pg, nxt)
                        pg_q[d].append(pg)
                if n_items:
                    want = min(n_items, (n_items * (tl + 2)) // (TC - 1))
                    while emitted < want:
                        items[emitted]()
                        emitted += 1
                if tl == TC - 1:
                    for d in range(2):
                        nc.sync.dma_start(out_d.ap()[d, c], hstages[d][:])

    nc.compile()
    return nc


_CACHED_NC = None


def _get_nc():
    global _CACHED_NC
    if _CACHED_NC is None:
        _CACHED_NC = build_nc()
    return _CACHED_NC


def _prep_inputs(video_feats, W_e, b_e, W_ih1, W_hh1, b_ih1, b_hh1,
                 W_ih2, W_hh2, b_ih2, b_hh2):
    # gate row scaling (sigmoid-via-tanh): i,f,o rows 0.5; g rows 1.0
    s = np.ones((4 * H,), np.float32)
    s[0 * H : 2 * H] = 0.5
    s[3 * H : 4 * H] = 0.5
    perm = np.concatenate(
        [
            np.arange(0 * H, 1 * H),  # i
            np.arange(2 * H, 3 * H),  # g
            np.arange(1 * H, 2 * H),  # f
            np.arange(3 * H, 4 * H),  # o
        ]
    )

    wet = np.ascontiguousarray(
        W_e.T.astype(np.float16).reshape(KF, 128, P).transpose(1, 0, 2)
    )
    bet = np.ascontiguousarray(b_e.reshape(KP, 128).T).astype(np.float32)

    def pack8(Wt, scale):
        Wq = (Wt * scale).astype(F8_NP)
        out = np.zeros((128, 2, NI, 2, 128), F8_NP)
        for mi, m in enumerate(IFO):
            blk = Wq[:, m * 128 : (m + 1) * 128]
            for pr in range(Wt.shape[0] // 256):
                for j in range(2):
                    kt = 2 * pr + j
                    out[:, pr, mi, j, :] = blk[kt * 128 : (kt + 1) * 128, :]
        return np.ascontiguousarray(out)

    per_dir = []
    for (W_ih, W_hh, b_ih, b_hh) in (
        (W_ih1, W_hh1, b_ih1, b_hh1),
        (W_ih2, W_hh2, b_ih2, b_hh2),
    ):
        wih_eff = ((W_ih * s[:, None])[perm]).T.astype(np.float32)
        whh_eff = ((W_hh * s[:, None] * 0.5)[perm]).T.astype(np.float32)
        bb = (((b_ih + b_hh) * s)[perm]).astype(np.float32)
        biasbc = np.broadcast_to(
            (np.ascontiguousarray(bb.reshape(MG, 128).T) * S_G)[:, :, None],
            (128, MG, BC),
        ).astype(np.float16)
        wihg = np.ascontiguousarray(
            (wih_eff * S_G).astype(np.float16)
            .reshape(KP, 128, 4 * H)
            .transpose(1, 0, 2)
        )
        whh8 = pack8(whh_eff, S_G)
        per_dir.append((wihg, whh8, np.ascontiguousarray(biasbc)))

    vt_full = np.ascontiguousarray(video_feats.transpose(2, 1, 0)).astype(np.float16)

    in_maps = []
    for core in range(8):
        vt = np.ascontiguousarray(
            vt_full[:, :, core * BC : (core + 1) * BC]
            .reshape(KF, 128, NCHUNK, TC, BC)
            .transpose(2, 1, 0, 3, 4)
        )
        im = {
            "vt": vt,
            "w_et": wet,
            "b_e_t": bet,
            "ident": np.eye(128, dtype=np.float16),
        }
        for d in range(2):
            wihg, whh8, biasbc = per_dir[d]
            im[f"w_ihg{d}"] = wihg
            im[f"w_hh8{d}"] = whh8
            im[f"biasbc{d}"] = biasbc
        in_maps.append(im)
    return in_maps


last_exec_ns = None
last_wall_s = None


def kernel(**inputs):
    global last_exec_ns, last_wall_s
    nc = _get_nc()
    inputs = {k: np.asarray(v, dtype=np.float32) for k, v in inputs.items()}
    in_maps = _prep_inputs(**inputs)
    t0 = time.perf_counter()
    res = run_bass_kernel_spmd(nc, in_maps, core_ids=list(range(8)))
    last_wall_s = time.perf_counter() - t0
    last_exec_ns = res.exec_time_ns

    lstm1 = np.empty((B, T, H), np.float32)
    lstm2 = np.empty((B, T, H), np.float32)
    for core in range(8):
        oh = res.results[core]["out_h"]  # [2, NCHUNK, 128, TC, KH, BC], holds 2h
        for d, dst in ((0, lstm1), (1, lstm2)):
            h = np.transpose(
                oh[d].astype(np.float32), (4, 0, 2, 3, 1)
            ).reshape(BC, T, H)
            h *= 0.5
            if d == 1:
                h = h[:, ::-1, :]
            dst[core * BC : (core + 1) * BC] = h
    return (lstm1, lstm2)
